# revision 19
# baseline (speedup 1.0000x reference)
"""CoxTime loss kernel for 8 Trainium2 NeuronCores.

Strategy (data-parallel over B):
  Each core reduces its (32768, 128) f32 logits shard to a (128, 256)
  summary using the TensorEngine with an on-the-fly one-hot of labels:
      S[c, k] = sum_{j: label_j == c} exp(logits[j, k])
      T[c, k] = sum_{j: label_j == c} ev_j * logits[j, k]
  The host all-reduces the 8 summaries and finishes:
      sumexp[k] = sum_{c >= k} S[c, k]        (risk-set mask is triangular
                                               in label-bin space)
      numer[k]  = T[k, k]
      n_ev, the log and the scalar reduction are O(K)/O(B-1d) host work.
"""

import numpy as np

import concourse.bacc as bacc
import concourse.bass as bass
import concourse.mybir as mybir
import concourse.tile as tile
from concourse.bass_utils import run_bass_kernel_spmd

B = 262144
K = 128
NCORES = 8
BC = B // NCORES  # rows per core
P = 128           # partitions (rows per tile)
TPB = 4           # row-tiles per DMA'd big tile

f32 = mybir.dt.float32
bf16 = mybir.dt.bfloat16
i32 = mybir.dt.int32
NBANK = 4  # alternating PSUM banks for matmul ILP

LAST_EXEC_NS = None
LAST_TRACE = None
LAST_PROFILE_JSON = None


def build_nc(bc=BC):
    """Build the per-core Bass program. bc = rows handled by this core."""
    nt = bc // P          # 128-row tiles
    nbig = nt // TPB      # big tiles per core
    assert nt * P == bc and nbig * TPB == nt

    nc = bacc.Bacc("TRN2", target_bir_lowering=False)
    logits = nc.declare_dram_parameter("logits", [bc, K], f32, isOutput=False)
    labcols = nc.declare_dram_parameter("labcols", [P, nt], f32, isOutput=False)
    evcols = nc.declare_dram_parameter("evcols", [P, nt], f32, isOutput=False)
    out = nc.declare_dram_parameter("out", [P, NBANK * 2 * K], f32,
                                    isOutput=True)

    with tile.TileContext(nc) as tc:
        with (
            tc.tile_pool(name="const", bufs=1) as cpool,
            tc.tile_pool(name="lt", bufs=6) as ltpool,
            tc.tile_pool(name="rhs", bufs=4) as rhspool,
            tc.tile_pool(name="oh", bufs=4) as ohpool,
            tc.tile_pool(name="psum", bufs=1, space="PSUM") as pspool,
        ):
            labc_f = cpool.tile([P, nt], f32)
            nc.sync.dma_start(out=labc_f[:], in_=labcols.ap())
            labc = cpool.tile([P, nt], bf16)
            nc.vector.tensor_copy(labc[:], labc_f[:])
            evc = cpool.tile([P, nt], f32)
            nc.sync.dma_start(out=evc[:], in_=evcols.ap())

            # iota over the k axis, replicated TPB times: [0..K-1]*TPB
            iota_i = cpool.tile([P, TPB * K], i32)
            nc.gpsimd.iota(iota_i[:], pattern=[[0, TPB], [1, K]], base=0,
                           channel_multiplier=0)
            iota_f = cpool.tile([P, TPB * K], bf16)
            nc.vector.tensor_copy(iota_f[:], iota_i[:])
            iota_f3 = iota_f[:].rearrange("p (q k) -> p q k", k=K)

            psums = [pspool.tile([P, 2 * K], f32, name=f"ps{b}", tag=f"ps{b}")
                     for b in range(NBANK)]

            lg3 = logits.ap().rearrange("(g q p) k -> g p q k", p=P, q=TPB)

            dma_engines = [nc.sync, nc.gpsimd, nc.scalar]
            for g in range(nbig):
                lt = ltpool.tile([P, TPB * K], f32)
                lt3 = lt[:].rearrange("p (q k) -> p q k", k=K)
                dma_engines[g % len(dma_engines)].dma_start(
                    out=lt3, in_=lg3[g])

                rhs = rhspool.tile([P, TPB * 2 * K], bf16)
                rhs3 = rhs[:].rearrange("p (q m) -> p q m", m=2 * K)

                # E = exp(logits) into the left half of each tile's rhs block
                nc.scalar.activation(out=rhs3[:, :, 0:K], in_=lt3,
                                     func=mybir.ActivationFunctionType.Exp)
                # ev * logits into the right half (also casts to bf16)
                ev_b = evc[:, g * TPB:(g + 1) * TPB][:, :, None].to_broadcast(
                    [P, TPB, K])
                nc.vector.tensor_tensor(out=rhs3[:, :, K:2 * K], in0=lt3,
                                        in1=ev_b, op=mybir.AluOpType.mult)

                # one-hot of labels: oh[p, q, k] = (label[t*128+p] == k)
                oh = ohpool.tile([P, TPB * K], bf16)
                oh3 = oh[:].rearrange("p (q k) -> p q k", k=K)
                lab_b = labc[:, g * TPB:(g + 1) * TPB][:, :, None].to_broadcast(
                    [P, TPB, K])
                nc.vector.tensor_tensor(out=oh3, in0=lab_b, in1=iota_f3,
                                        op=mybir.AluOpType.is_equal)

                for q in range(TPB):
                    t = g * TPB + q
                    b = t % NBANK
                    nc.tensor.matmul(
                        out=psums[b][:],
                        lhsT=oh[:, q * K:(q + 1) * K],
                        rhs=rhs[:, q * 2 * K:(q + 1) * 2 * K],
                        start=(t < NBANK),
                        stop=(t >= nt - NBANK),
                    )

            osb = cpool.tile([P, NBANK * 2 * K], f32)
            for b in range(NBANK):
                nc.vector.tensor_copy(
                    osb[:, b * 2 * K:(b + 1) * 2 * K], psums[b][:])
            nc.sync.dma_start(out=out.ap(), in_=osb[:])

    nc.compile()
    return nc


def _shard_inputs(logits, labels, events):
    """Build the 8 per-core input maps (host-side layout only)."""
    logits = np.ascontiguousarray(np.asarray(logits, dtype=np.float32))
    labels = np.asarray(labels, dtype=np.int32)
    events = np.asarray(events, dtype=np.int32)
    nt = BC // P
    in_maps = []
    for i in range(NCORES):
        sl = slice(i * BC, (i + 1) * BC)
        lab = labels[sl].astype(np.float32).reshape(nt, P).T
        ev = (events[sl] == 1).astype(np.float32).reshape(nt, P).T
        in_maps.append({
            "logits": logits[sl],
            "labcols": np.ascontiguousarray(lab),
            "evcols": np.ascontiguousarray(ev),
        })
    return in_maps


def _finish(outs, labels, events):
    """Host epilogue: all-reduce summaries, triangular sum, log, scalar."""
    labels = np.asarray(labels, dtype=np.int32)
    events = np.asarray(events, dtype=np.int32)
    acc = np.zeros((P, NBANK, 2 * K), dtype=np.float64)
    for o in outs:
        acc += o.astype(np.float64).reshape(P, NBANK, 2 * K)
    acc = acc.sum(axis=1)
    S = acc[:, :K]          # S[c, k]
    T = acc[:, K:]
    # sumexp[k] = sum over label bins c >= k
    sumexp = (S * np.tri(K)).sum(axis=0)
    numer = np.diag(T)
    n_ev = np.bincount(labels[events == 1], minlength=K).astype(np.float64)
    with np.errstate(divide="ignore"):
        denom_log = np.log(sumexp)
    terms = np.where(n_ev > 0, numer - n_ev * denom_log, 0.0)
    n_total = max(n_ev.sum(), 1.0)
    return np.array(-terms.sum() / n_total, dtype=np.float32)


def kernel(logits, labels, events, _trace=False):
    global LAST_EXEC_NS, LAST_TRACE, LAST_PROFILE_JSON
    in_maps = _shard_inputs(logits, labels, events)
    nc = build_nc()
    res = run_bass_kernel_spmd(nc, in_maps, core_ids=list(range(NCORES)),
                               trace=_trace)
    LAST_EXEC_NS = res.exec_time_ns
    LAST_TRACE = res.instructions_and_trace
    LAST_PROFILE_JSON = res.profile_json
    outs = [res.results[i]["out"] for i in range(NCORES)]
    return _finish(outs, labels, events)


# revision 22
# speedup vs baseline: 1.0460x; 1.0460x over previous
"""CoxTime loss kernel for 8 Trainium2 NeuronCores.

Strategy (data-parallel over B):
  Each core reduces its (32768, 128) f32 logits shard to a (128, 256)
  summary using the TensorEngine with an on-the-fly one-hot of labels:
      S[c, k] = sum_{j: label_j == c} exp(logits[j, k])
      T[c, k] = sum_{j: label_j == c} ev_j * logits[j, k]
  The host all-reduces the 8 summaries and finishes:
      sumexp[k] = sum_{c >= k} S[c, k]        (risk-set mask is triangular
                                               in label-bin space)
      numer[k]  = T[k, k]
      n_ev, the log and the scalar reduction are O(K)/O(B-1d) host work.
"""

import numpy as np

import concourse.bacc as bacc
import concourse.bass as bass
import concourse.mybir as mybir
import concourse.tile as tile
from concourse.bass_utils import run_bass_kernel_spmd

B = 262144
K = 128
NCORES = 8
BC = B // NCORES  # rows per core
P = 128           # partitions (rows per tile)
TPB = 8           # row-tiles per DMA'd big tile

f32 = mybir.dt.float32
bf16 = mybir.dt.bfloat16
i32 = mybir.dt.int32
NBANK = 4  # alternating PSUM banks for matmul ILP

LAST_EXEC_NS = None
LAST_TRACE = None
LAST_PROFILE_JSON = None


def build_nc(bc=BC):
    """Build the per-core Bass program. bc = rows handled by this core."""
    nt = bc // P          # 128-row tiles
    nbig = nt // TPB      # big tiles per core
    assert nt * P == bc and nbig * TPB == nt

    nc = bacc.Bacc("TRN2", target_bir_lowering=False)
    logits = nc.declare_dram_parameter("logits", [bc, K], f32, isOutput=False)
    labcols = nc.declare_dram_parameter("labcols", [P, nt], f32, isOutput=False)
    evcols = nc.declare_dram_parameter("evcols", [P, nt], f32, isOutput=False)
    out = nc.declare_dram_parameter("out", [P, NBANK * 2 * K], f32,
                                    isOutput=True)

    with tile.TileContext(nc) as tc:
        with (
            tc.tile_pool(name="const", bufs=1) as cpool,
            tc.tile_pool(name="lt", bufs=4) as ltpool,
            tc.tile_pool(name="rhs", bufs=3) as rhspool,
            tc.tile_pool(name="oh", bufs=3) as ohpool,
            tc.tile_pool(name="psum", bufs=1, space="PSUM") as pspool,
        ):
            labc_f = cpool.tile([P, nt], f32)
            nc.sync.dma_start(out=labc_f[:], in_=labcols.ap())
            labc = cpool.tile([P, nt], bf16)
            nc.vector.tensor_copy(labc[:], labc_f[:])
            evc = cpool.tile([P, nt], f32)
            nc.sync.dma_start(out=evc[:], in_=evcols.ap())

            # iota over the k axis, replicated TPB times: [0..K-1]*TPB
            iota_i = cpool.tile([P, TPB * K], i32)
            nc.gpsimd.iota(iota_i[:], pattern=[[0, TPB], [1, K]], base=0,
                           channel_multiplier=0)
            iota_f = cpool.tile([P, TPB * K], bf16)
            nc.vector.tensor_copy(iota_f[:], iota_i[:])
            iota_f3 = iota_f[:].rearrange("p (q k) -> p q k", k=K)

            psums = [pspool.tile([P, 2 * K], f32, name=f"ps{b}", tag=f"ps{b}")
                     for b in range(NBANK)]

            lg3 = logits.ap().rearrange("(g q p) k -> g p q k", p=P, q=TPB)

            dma_engines = [nc.sync, nc.gpsimd, nc.scalar]
            for g in range(nbig):
                lt = ltpool.tile([P, TPB * K], f32)
                lt3 = lt[:].rearrange("p (q k) -> p q k", k=K)
                dma_engines[g % len(dma_engines)].dma_start(
                    out=lt3, in_=lg3[g])

                rhs = rhspool.tile([P, TPB * 2 * K], bf16)
                rhs3 = rhs[:].rearrange("p (q m) -> p q m", m=2 * K)

                # E = exp(logits) into the left half of each tile's rhs block
                nc.scalar.activation(out=rhs3[:, :, 0:K], in_=lt3,
                                     func=mybir.ActivationFunctionType.Exp)
                # ev * logits into the right half (also casts to bf16)
                ev_b = evc[:, g * TPB:(g + 1) * TPB][:, :, None].to_broadcast(
                    [P, TPB, K])
                nc.vector.tensor_tensor(out=rhs3[:, :, K:2 * K], in0=lt3,
                                        in1=ev_b, op=mybir.AluOpType.mult)

                # one-hot of labels: oh[p, q, k] = (label[t*128+p] == k)
                oh = ohpool.tile([P, TPB * K], bf16)
                oh3 = oh[:].rearrange("p (q k) -> p q k", k=K)
                lab_b = labc[:, g * TPB:(g + 1) * TPB][:, :, None].to_broadcast(
                    [P, TPB, K])
                nc.vector.tensor_tensor(out=oh3, in0=iota_f3, in1=lab_b,
                                        op=mybir.AluOpType.is_equal)

                for q in range(TPB):
                    t = g * TPB + q
                    b = t % NBANK
                    nc.tensor.matmul(
                        out=psums[b][:],
                        lhsT=oh[:, q * K:(q + 1) * K],
                        rhs=rhs[:, q * 2 * K:(q + 1) * 2 * K],
                        start=(t < NBANK),
                        stop=(t >= nt - NBANK),
                    )

            osb = cpool.tile([P, NBANK * 2 * K], f32)
            for b in range(NBANK):
                nc.vector.tensor_copy(
                    osb[:, b * 2 * K:(b + 1) * 2 * K], psums[b][:])
            nc.sync.dma_start(out=out.ap(), in_=osb[:])

    nc.compile()
    return nc


def _shard_inputs(logits, labels, events):
    """Build the 8 per-core input maps (host-side layout only)."""
    logits = np.ascontiguousarray(np.asarray(logits, dtype=np.float32))
    labels = np.asarray(labels, dtype=np.int32)
    events = np.asarray(events, dtype=np.int32)
    nt = BC // P
    in_maps = []
    for i in range(NCORES):
        sl = slice(i * BC, (i + 1) * BC)
        lab = labels[sl].astype(np.float32).reshape(nt, P).T
        ev = (events[sl] == 1).astype(np.float32).reshape(nt, P).T
        in_maps.append({
            "logits": logits[sl],
            "labcols": np.ascontiguousarray(lab),
            "evcols": np.ascontiguousarray(ev),
        })
    return in_maps


def _finish(outs, labels, events):
    """Host epilogue: all-reduce summaries, triangular sum, log, scalar."""
    labels = np.asarray(labels, dtype=np.int32)
    events = np.asarray(events, dtype=np.int32)
    acc = np.zeros((P, NBANK, 2 * K), dtype=np.float64)
    for o in outs:
        acc += o.astype(np.float64).reshape(P, NBANK, 2 * K)
    acc = acc.sum(axis=1)
    S = acc[:, :K]          # S[c, k]
    T = acc[:, K:]
    # sumexp[k] = sum over label bins c >= k
    sumexp = (S * np.tri(K)).sum(axis=0)
    numer = np.diag(T)
    n_ev = np.bincount(labels[events == 1], minlength=K).astype(np.float64)
    with np.errstate(divide="ignore"):
        denom_log = np.log(sumexp)
    terms = np.where(n_ev > 0, numer - n_ev * denom_log, 0.0)
    n_total = max(n_ev.sum(), 1.0)
    return np.array(-terms.sum() / n_total, dtype=np.float32)


def kernel(logits, labels, events, _trace=False):
    global LAST_EXEC_NS, LAST_TRACE, LAST_PROFILE_JSON
    in_maps = _shard_inputs(logits, labels, events)
    nc = build_nc()
    res = run_bass_kernel_spmd(nc, in_maps, core_ids=list(range(NCORES)),
                               trace=_trace)
    LAST_EXEC_NS = res.exec_time_ns
    LAST_TRACE = res.instructions_and_trace
    LAST_PROFILE_JSON = res.profile_json
    outs = [res.results[i]["out"] for i in range(NCORES)]
    return _finish(outs, labels, events)
